# revision 97
# baseline (speedup 1.0000x reference)
"""Multi-head causal attention (b=4, n=2048, d=1024, h=16) on 8 trn2 cores.

Sharding: data-parallel over batch (4) x tensor-parallel over heads (2 groups
of 8 heads).  Core c handles batch c//2, heads 8*(c%2)..8*(c%2)+8.

Per-core dataflow (PE cost model: matmul = moving-cols * rate; bf16 rate 1,
fp8 DoubleRow rate 0.5 with 256-deep contraction per instruction):

  Projections (fp8e4 DoubleRow, 3-term residual split for accuracy):
    Q' = x8@W8 + r8@W8 + x8@S8   where x = x8+r8 (e4m3 + e4m3 residual),
    128*W.T = W8+S8.  12 DR matmuls per [128,512] output tile (vs 8 bf16
    full-K matmuls) at half rate -> 0.75x bf16 cost, ~bf16 accuracy.
    KT [128=pair dims, 4, 2048] bf16; VP [128=tok, 16, 8, 65] bf16 (ones col
    for softmax sums); QT per strip [128, 4, 512] bf16.
  S^T blocks [128 keys, <=512 queries] = KT_blk.T @ QT (bf16, K=64, two heads
    of a pair on PE row halves 0-63/64-127); diagonal blocks column-trimmed
    (causal), masked-after-exp by a 0/1 triangle multiply on DVE (bf16, 2x).
  exp on ACT: [128, 2, 512] PSUM chunks -> ptc bf16, scale 2^-17 (folds the
    1/sqrt(64) and the 128^2 weight prescale).
  PV transposed: out[q 128, 65] += ptc_blk[128k,128q].T @ VP_blk[128k,65]:
    65 moving cols per block (vs 512 in the untransposed orientation).
    Softmax normalize = DVE tensor_scalar divide by the per-partition sums
    column (+ 1/128 weight-scale fold), into a [128, 8, 64] f32 stage tile;
    one SWDGE DMA per 128-query row block.

Schedule: single global weave queue of projection half-groups + pending PV
groups, pumped between QK/exp chunk emissions to keep PE ahead of ACT (ACT
exp is the roofline: ~123us busy minimum for 8 heads causal n=2048).
"""

import numpy as np
import ml_dtypes

import concourse.bacc as bacc
import concourse.mybir as mybir
import concourse.tile as tile
from concourse import bass_utils
from concourse.bass_interp import get_hw_module

N_CORES = 8
B, N, D = 4, 2048, 1024
HEADS = 16
HPC = 8            # heads per core
HD = 64            # head dim
GD = HPC * HD      # 512 weight columns per core
NB = N // 128      # 16 key blocks
NSTRIP = N // 512  # 4 query strips
WSCALE = 128.0     # weight prescale so fp8 residuals clear e4m3 subnormals

f32 = mybir.dt.float32
bf16 = mybir.dt.bfloat16
f8 = mybir.dt.float8e4
EXP = mybir.ActivationFunctionType.Exp
DR = mybir.MatmulPerfMode.DoubleRow
EXP_SCALE = 0.125 / (WSCALE * WSCALE)  # 2^-17, exact in f32


def build_program():
    nc = bacc.Bacc("TRN2", target_bir_lowering=False, debug=False,
                   num_devices=N_CORES)
    # host-pretiled layouts: x [strip][p][c][i][t]; q/k weights
    # [m][p][c][i][f] (m-granular so the first feat-block lands fast);
    # v weights [p][c][i][f] (f-contiguous for the 512-wide moving operand)
    x8d = nc.dram_tensor("x8", [4, 128, 4, 2, 512], f8,
                         kind="ExternalInput").ap()
    xr8d = nc.dram_tensor("xr8", [4, 128, 4, 2, 512], f8,
                          kind="ExternalInput").ap()
    wd = {}
    for nm in ("q", "k", "qs", "ks"):
        wd[nm] = nc.dram_tensor(f"w8{nm}", [4, 128, 4, 2, 128], f8,
                                kind="ExternalInput").ap()
    for nm in ("v", "vs"):
        wd[nm] = nc.dram_tensor(f"w8{nm}", [128, 4, 2, GD], f8,
                                kind="ExternalInput").ap()
    trid = nc.dram_tensor("tri01", [128, 128], bf16, kind="ExternalInput").ap()
    outN = nc.dram_tensor("outN", [N, GD], f32, kind="ExternalOutput").ap()

    with tile.TileContext(nc) as tc:
        with (
            tc.tile_pool(name="w", bufs=1) as w_pool,
            tc.tile_pool(name="xs", bufs=2) as xs_pool,
            tc.tile_pool(name="big", bufs=1) as big_pool,
            tc.tile_pool(name="qt", bufs=2) as qt_pool,
            tc.tile_pool(name="ptc", bufs=3) as ptc_pool,
            tc.tile_pool(name="stage", bufs=8) as stage_pool,
            tc.tile_pool(name="small", bufs=1) as small_pool,
            tc.tile_pool(name="sums", bufs=4) as sums_pool,
            tc.tile_pool(name="psp", bufs=2, space="PSUM") as psp_pool,
            tc.tile_pool(name="pss", bufs=2, space="PSUM") as pss_pool,
            tc.tile_pool(name="po", bufs=2, space="PSUM") as po_pool,
        ):
            # ---- weights + strip-0 x: DMA ordered by first use; q/k weights
            # are m-granular so KT/QT feat-block 0 can start after ~2.5us
            wt = {}
            for nm in ("k", "ks", "q", "qs"):
                wt[nm] = w_pool.tile([128, 4, 4, 2, 128], f8, tag=f"w{nm}",
                                     name=f"w{nm}")
            for nm in ("v", "vs"):
                wt[nm] = w_pool.tile([128, 4, 2, GD], f8, tag=f"w{nm}",
                                     name=f"w{nm}")

            def dma_x(eng, xtile, dsrc, n):
                eng.dma_start(xtile[:], dsrc[n])

            # transfer order == demand order. The DMA engines serve transfers
            # in descriptor-gen completion order, so everything goes through
            # HWDGE (SWDGE gens finish early and queue-jump) with issue order
            # exactly matching first use: KT0/QT0 main terms, then residuals,
            # then V weights, then feat-blocks 1-3.
            # single-queue prologue: HWDGE gens (and therefore transfers)
            # serve strictly in issue order only within one queue — cross-
            # queue gen interleaving otherwise lets late-need tensors (V
            # weights, x-residual) queue-jump the QT0-critical w8q transfer
            xs8 = xs_pool.tile([128, 4, 2, 512], f8, tag="xs8", name="xs8_0")
            xr8 = xs_pool.tile([128, 4, 2, 512], f8, tag="xr8", name="xr8_0")
            nc.sync.dma_start(wt["k"][:, 0], wd["k"][0])
            dma_x(nc.sync, xs8, x8d, 0)
            nc.sync.dma_start(wt["q"][:, 0], wd["q"][0])
            nc.sync.dma_start(wt["ks"][:, 0], wd["ks"][0])
            nc.sync.dma_start(wt["qs"][:, 0], wd["qs"][0])
            dma_x(nc.sync, xr8, xr8d, 0)
            tri = small_pool.tile([128, 128], bf16, tag="tri")
            nc.sync.dma_start(tri[:], trid[:])
            # feat-block 1 weights gate pair (0,1) before the V weights are
            # first read (VP copies land during pair 1's PV consumers)
            for nm in ("k", "ks", "q", "qs"):
                nc.sync.dma_start(wt[nm][:, 1], wd[nm][1])
            nc.sync.dma_start(wt["v"][:], wd["v"][:])
            nc.sync.dma_start(wt["vs"][:], wd["vs"][:])
            for m in range(2, 4):
                for nm in ("k", "ks", "q", "qs"):
                    nc.sync.dma_start(wt[nm][:, m], wd[nm][m])

            kt = big_pool.tile([128, 4, N], bf16, tag="kt")
            vp = big_pool.tile([128, NB, HPC, HD + 1], bf16, tag="vp")
            # sums column: WSCALE (not 1.0) so po[:,64] = WSCALE*sums and a
            # single reciprocal+mul folds away the weight prescale
            nc.vector.memset(vp[:, :, :, HD:HD + 1], WSCALE)
            # warm the ACT exp table while input DMAs stream
            warmup = small_pool.tile([1, 1], f32, tag="warmup")
            nc.vector.memset(warmup[:], 0.0)
            nc.scalar.activation(warmup[:], warmup[:], EXP)

            # ---- projection machinery -------------------------------------
            # Each [128, 512] output tile accumulates 12 DR matmuls
            # (4 k-chunks x 3 split terms), emitted as two 6-matmul
            # half-groups for finer weave granularity.
            def proj_halves(wa, wb, xa, xb, copy_fn, swap):
                """wa/wb: weight main+residual APs fn(c)->[128,2,*];
                xa/xb: x main+residual fn(c); swap: x is the stationary side
                (V projection).  Returns two closures."""
                st = {}

                def terms():
                    # term-major so main*main runs before residual DMAs land
                    lst = []
                    for wf, xf in ((wa, xa), (wb, xa), (wa, xb)):
                        for c in range(4):
                            lst.append((xf(c), wf(c)) if swap
                                       else (wf(c), xf(c)))
                    return lst

                def half(first):
                    def run():
                        if first:
                            st["ps"] = psp_pool.tile([128, 512], f32,
                                                     tag="psp", name="psp")
                        ps = st["ps"]
                        lo, hi = (0, 6) if first else (6, 12)
                        for idx, (lhsT, rhs) in enumerate(terms()[lo:hi], lo):
                            nc.tensor.matmul(ps[:], lhsT, rhs,
                                             start=(idx == 0),
                                             stop=(idx == 11),
                                             perf_mode=DR)
                        if not first:
                            copy_fn(ps)
                    return run
                return [half(True), half(False)]

            def emit_strip_groups(n, xs8, xr8, qts):
                """Queue items for strip n: (kind, key, fn) where kind 'kq'
                needs m<=pair before pair, 'vp' needs i before PV group i."""
                items = []
                for m in range(4):
                    for nm, dst in (("k", "kt"), ("q", "qt")):
                        if dst == "kt" and n == 0 and m == 0:
                            # first KT copy on the (idle) ACT engine so it
                            # runs parallel to QT0's DVE copy — both gate
                            # the very first QK chunk
                            cp = (lambda ps, m=m: nc.scalar.copy(
                                kt[:, m, n * 512:(n + 1) * 512], ps[:]))
                        elif dst == "kt":
                            cp = (lambda ps, m=m: nc.vector.tensor_copy(
                                kt[:, m, n * 512:(n + 1) * 512], ps[:]))
                        else:
                            cp = (lambda ps, m=m: nc.vector.tensor_copy(
                                qts[:, m, :], ps[:]))
                        halves = proj_halves(
                            lambda c, nm=nm, m=m: wt[nm][:, m, c, :, :],
                            lambda c, nm=nm, m=m: wt[nm + "s"][:, m, c, :, :],
                            lambda c: xs8[:, c, :, :],
                            lambda c: xr8[:, c, :, :],
                            cp,
                            swap=False)
                        for h in halves:
                            items.append(("kq", m, h))
                for i in range(4):
                    blk = 4 * n + i
                    sl = slice(128 * i, 128 * (i + 1))
                    halves = proj_halves(
                        lambda c: wt["v"][:, c, :, :],
                        lambda c: wt["vs"][:, c, :, :],
                        lambda c, sl=sl: xs8[:, c, :, sl],
                        lambda c, sl=sl: xr8[:, c, :, sl],
                        lambda ps, blk=blk: nc.vector.tensor_copy(
                            vp[:, blk, :, 0:HD],
                            ps[:].rearrange("p (h d) -> p h d", h=HPC)),
                        swap=True)
                    for h in halves:
                        items.append(("vp", i, h))
                # order: KT0 QT0 KT1 QT1 | VP0 VP1 | KT2 QT2 | VP2 VP3 |
                # KT3 QT3 — V projections deferred to just before first use
                kq = [items[4 * m:4 * m + 4] for m in range(4)]
                vps = [items[16 + 2 * i:16 + 2 * i + 2] for i in range(4)]
                return (kq[0] + kq[1] + vps[0] + vps[1] + kq[2] + vps[2] +
                        vps[3] + kq[3])

            # ---- global weave state ---------------------------------------
            queue = []       # [(strip, kind, key, fn)]
            pending = []     # [(gid, ready_chunk, fn)] PV groups
            chunks_done = {}  # (strip, pair) -> chunks emitted
            stage_st = {}    # (strip, i) -> [tile, writes]
            pv_alt = {"on": False, "n": 0}

            def pop_queue_until(pred):
                i = 0
                while i < len(queue):
                    if pred(queue[i]):
                        queue.pop(i)[3]()
                    else:
                        i += 1

            def pump(credits, cur_gid, cur_chunks, queue_first=False):
                while credits > 0:
                    if queue_first and queue:
                        queue.pop(0)[3]()
                        credits -= 1
                        continue
                    did = False
                    for i, (gid, rdy, fn) in enumerate(pending):
                        if gid < cur_gid or rdy <= cur_chunks:
                            pending.pop(i)
                            fn()
                            credits -= 1
                            did = True
                            break
                    if did:
                        continue
                    if queue:
                        queue.pop(0)[3]()
                        credits -= 1
                    else:
                        break

            def flush_pending_before(gid_limit):
                i = 0
                while i < len(pending):
                    if pending[i][0] < gid_limit:
                        pending.pop(i)[2]()
                    else:
                        i += 1

            def emit_pv_part(qs, h, i, ptc_pair, hh, state, j_hi):
                """Accumulate PV blocks j < j_hi into state['po']."""
                t = 4 * qs + i
                j_hi = min(j_hi, t + 1)
                if "po" not in state:
                    pop_queue_until(
                        lambda it, qs=qs, i=i: it[1] == "vp" and
                        (it[0] < qs or (it[0] == qs and it[2] <= i)))
                    # in the projection-free endgame, rotate PV accumulators
                    # through the idle proj-psum banks too (4-deep pipeline)
                    if pv_alt["on"] and pv_alt["n"] % 2:
                        state["po"] = psp_pool.tile([128, HD + 1], f32,
                                                    tag="psp", name="po",
                                                    padded_shape=[128, 512])
                    else:
                        state["po"] = po_pool.tile([128, HD + 1], f32,
                                                   tag="po", name="po")
                    pv_alt["n"] += 1
                    state["j"] = 0
                po = state["po"]
                for j in range(state["j"], j_hi):
                    nc.tensor.matmul(
                        po[:], ptc_pair[:, hh, j, 128 * i:128 * (i + 1)],
                        vp[:, j, h, :], start=(j == 0), stop=(j == t))
                state["j"] = j_hi

            def emit_pv_group(qs, h, i, ptc_pair, hh, state=None):
                if state is None:
                    state = {}
                emit_pv_part(qs, h, i, ptc_pair, hh, state, 4 * qs + i + 1)
                po = state["po"]
                key = (qs, i)
                if key not in stage_st:
                    stage_st[key] = [stage_pool.tile(
                        [128, HPC, HD], f32, tag="stage",
                        name=f"stage_{qs}_{i}"), 0]
                st = stage_st[key]
                # out = po[:,0:64] * (1 / (WSCALE*sums)); col 64 = WSCALE*sums
                rec = sums_pool.tile([128, 1], f32, tag="rec", name="rec")
                nc.vector.reciprocal(rec[:], po[:, HD:HD + 1])
                nc.vector.tensor_scalar_mul(st[0][:, h, :], po[:, 0:HD],
                                            rec[:])
                st[1] += 1
                if st[1] % 2 == 0:
                    # both heads of pair p landed: ship their 128 columns now
                    # (keeps the final DMA tail to one small transfer)
                    p = st[1] // 2 - 1
                    rows = slice(512 * qs + 128 * i, 512 * qs + 128 * (i + 1))
                    eng = nc.sync
                    eng.dma_start(
                        outN[rows, 128 * p:128 * (p + 1)],
                        st[0][:, 2 * p:2 * p + 2, :]
                        .rearrange("p h d -> p (h d)"))
                    if st[1] == HPC:
                        del stage_st[key]

            # ---- main emission --------------------------------------------
            qts = qt_pool.tile([128, 4, 512], bf16, tag="qts", name="qts0")
            queue.extend((0, k, key, fn)
                         for k, key, fn in emit_strip_groups(0, xs8, xr8, qts))
            # pair-0 prerequisites upfront: KT0 + QT0 (both halves)
            for _ in range(4):
                queue.pop(0)[3]()

            for qs in range(NSTRIP):
                if qs + 1 < NSTRIP:
                    xs8_n = xs_pool.tile([128, 4, 2, 512], f8, tag="xs8",
                                         name=f"xs8_{qs + 1}")
                    xr8_n = xs_pool.tile([128, 4, 2, 512], f8, tag="xr8",
                                         name=f"xr8_{qs + 1}")
                    dma_x(nc.sync, xs8_n, x8d, qs + 1)
                    dma_x(nc.sync, xr8_n, xr8d, qs + 1)
                    qts_n = qt_pool.tile([128, 4, 512], bf16, tag="qts",
                                         name=f"qts{qs + 1}")
                    queue.extend(
                        (qs + 1, k, key, fn) for k, key, fn in
                        emit_strip_groups(qs + 1, xs8_n, xr8_n, qts_n))

                nblocks = 4 * qs + 4
                for p in range(4):
                    gid = 4 * qs + p
                    # prerequisites: KT/QT feat-block <=p of this and all
                    # earlier strips; PV groups older than the previous pair
                    if gid == 14:
                        # endgame: drain all remaining projection groups so
                        # their psum banks are free for PV rotation
                        pop_queue_until(lambda it: True)
                        pv_alt["on"] = True
                    else:
                        pop_queue_until(
                            lambda it, qs=qs, p=p: it[0] <= qs and
                            it[1] == "kq" and it[2] <= p)
                    flush_pending_before(gid - 1)
                    ptc_pair = ptc_pool.tile([128, 2, NB, 512], bf16,
                                             tag="ptc", name=f"ptc_{gid}")
                    chunks_done[(qs, p)] = 0
                    for c in range(nblocks // 2):
                        jj = (2 * c, 2 * c + 1)
                        pss = {}
                        for hh in range(2):
                            p0 = 64 * hh
                            ps = pss_pool.tile([128, 2, 512], f32, tag="pss",
                                               name=f"pss{hh}")
                            pss[hh] = ps
                            for idx, j in enumerate(jj):
                                r = j - 4 * qs
                                c0 = 128 * r if r > 0 else 0
                                nc.tensor.matmul(
                                    ps[:, idx, c0:512],
                                    kt[p0:p0 + 64, p, 128 * j:128 * (j + 1)],
                                    qts[p0:p0 + 64, p, c0:512],
                                    start=True, stop=True)
                        r_lo = 2 * c - 4 * qs
                        ce = 128 * r_lo if r_lo > 0 else 0
                        for hh in range(2):
                            nc.scalar.activation(
                                ptc_pair[:, hh, 2 * c:2 * c + 2, ce:512],
                                pss[hh][:, 0:2, ce:512], EXP, scale=EXP_SCALE)
                            for idx, j in enumerate(jj):
                                r = j - 4 * qs
                                if r >= 0:
                                    nc.vector.tensor_mul(
                                        ptc_pair[:, hh, j,
                                                 128 * r:128 * (r + 1)],
                                        ptc_pair[:, hh, j,
                                                 128 * r:128 * (r + 1)],
                                        tri[:])
                        chunks_done[(qs, p)] = c + 1
                        credits = 2 if p < 3 else (3 if gid < 15 else 4)
                        pump(credits, gid, c + 1)
                    # queue this pair's PV groups; they weave into the next
                    # pair's chunk loop (ready once their chunks are exp'd).
                    # For the final pair, split groups needing the last chunk
                    # so only 1-2 matmuls sit on the tail critical path.
                    for i in range(4):
                        rdy = (4 * qs + i + 2) // 2
                        for hh in range(2):
                            h = 2 * p + hh
                            if gid == 15 and rdy >= nblocks // 2:
                                st_pv = {}
                                pending.append((
                                    gid, rdy - 1,
                                    lambda qs=qs, h=h, i=i, pp=ptc_pair,
                                    hh=hh, st_pv=st_pv, rdy=rdy:
                                    emit_pv_part(qs, h, i, pp, hh, st_pv,
                                                 2 * (rdy - 1))))
                                pending.append((
                                    gid, rdy,
                                    lambda qs=qs, h=h, i=i, pp=ptc_pair,
                                    hh=hh, st_pv=st_pv: emit_pv_group(
                                        qs, h, i, pp, hh, st_pv)))
                            else:
                                pending.append((
                                    gid, rdy,
                                    lambda qs=qs, h=h, i=i, ptc_pair=ptc_pair,
                                    hh=hh: emit_pv_group(qs, h, i, ptc_pair,
                                                         hh)))
                qts = qts_n if qs + 1 < NSTRIP else qts

            # epilogue: drain everything
            flush_pending_before(1 << 30)
            while queue:
                queue.pop(0)[3]()

    nc.compile()
    nc.m = get_hw_module(nc.m)
    return nc


_PROGRAM = None


def _program():
    global _PROGRAM
    if _PROGRAM is None:
        _PROGRAM = build_program()
    return _PROGRAM


def _q8(a):
    return np.ascontiguousarray(a).astype(ml_dtypes.float8_e4m3)


def make_in_maps(x, Wq, Wk, Wv):
    kk, qq = np.meshgrid(np.arange(128), np.arange(128), indexing="ij")
    tri = (qq >= kk).astype(ml_dtypes.bfloat16)  # [k, q] keep where q >= k
    x = np.asarray(x, np.float32)

    def tile_x(a):  # [1024, 2048] -> [strip, p, c, i, t]
        return np.ascontiguousarray(
            a.reshape(4, 2, 128, 4, 512).transpose(3, 2, 0, 1, 4))

    def tile_wqk(a):  # [1024, 512] -> [m, p, c, i, f]
        return np.ascontiguousarray(
            a.reshape(4, 2, 128, 4, 128).transpose(3, 2, 0, 1, 4))

    def tile_wv(a):  # [1024, 512] -> [p, c, i, f]
        return np.ascontiguousarray(
            a.reshape(4, 2, 128, 512).transpose(2, 0, 1, 3))

    in_maps = []
    for c in range(N_CORES):
        b, g = c // 2, c % 2
        sl = slice(g * GD, (g + 1) * GD)
        xT = np.ascontiguousarray(x[b].T)
        x8 = _q8(xT)
        xr8 = _q8(xT - x8.astype(np.float32))
        m = {"x8": tile_x(x8), "xr8": tile_x(xr8), "tri01": tri}
        for nm, W in (("q", Wq), ("k", Wk), ("v", Wv)):
            Wt = np.ascontiguousarray(np.asarray(W, np.float32).T[:, sl])
            Wt = Wt * WSCALE
            w8 = _q8(Wt)
            ws8 = _q8(Wt - w8.astype(np.float32))
            tw = tile_wv if nm == "v" else tile_wqk
            m[f"w8{nm}"] = tw(w8)
            m[f"w8{nm}s"] = tw(ws8)
        in_maps.append(m)
    return in_maps


def gather(results):
    out = np.empty((B, N, D), np.float32)
    for c in range(N_CORES):
        b, g = c // 2, c % 2
        out[b, :, g * GD:(g + 1) * GD] = results[c]["outN"]
    return out


def kernel(x, Wq, Wk, Wv):
    nc = _program()
    in_maps = make_in_maps(x, Wq, Wk, Wv)
    res = bass_utils.run_bass_kernel_spmd(nc, in_maps,
                                          core_ids=list(range(N_CORES)))
    return gather(res.results)
